# revision 2
# baseline (speedup 1.0000x reference)
"""Trainium2 Bass kernel for nn_IsingModel: one sequential Gibbs sweep.

Math (per independent chain):
    for j in 0..N-1:
        field_j = h_j + sum_k J[k, j] * s_k          (s = current spins)
        flip iff  -log(u_j) > s_j * field_j
        s_j *= -1 if flip

Sharding: the 200 chains (R*S) are split 25 per core across 8 cores;
chains are fully independent (zero communication).

Device layout (per core, phase 1 - simple DVE version):
    jm [N, CH, N]  f32 : jm[j, c, k] = J_sym[c, k, j] (= J_sym[c, j, k], symmetric)
    s0 [CH, N]     f32 : initial spins, chains on partitions
    rr [CH, N]     f32 : r_eff = -log(u) - s0*h  (h folded into threshold)
    so [CH, N]     f32 : output spins

Per node j (all DVE, chains on partitions [25, x]):
    field = accum_out( jm_slab_j * s_cur )                  (1 op, [25,360])
    phi   = (field * s_j) - r_j                             (tensor_scalar)
    sb    = (phi < 0) * s_j                                 (scalar_tensor_tensor)
    s_j   = (sb * -2) + s_j                                 (scalar_tensor_tensor)
"""

import sys

if "/opt/trn_rl_repo" not in sys.path:
    sys.path.insert(0, "/opt/trn_rl_repo")

from contextlib import ExitStack

import numpy as np

R, S, N = 10, 20, 360
NCORES = 8
CH = (R * S) // NCORES  # 25 chains per core

_cache = {}


def _build():
    import concourse.bass as bass
    import concourse.tile as tile
    from concourse import bacc, mybir

    f32 = mybir.dt.float32
    op = mybir.AluOpType

    nc = bacc.Bacc("TRN2", target_bir_lowering=False, debug=False)
    jm = nc.dram_tensor("jm", [N, CH, N], f32, kind="ExternalInput")
    s0 = nc.dram_tensor("s0", [CH, N], f32, kind="ExternalInput")
    rr = nc.dram_tensor("rr", [CH, N], f32, kind="ExternalInput")
    so = nc.dram_tensor("so", [CH, N], f32, kind="ExternalOutput")

    with tile.TileContext(nc) as tc, ExitStack() as ctx:
        singles = ctx.enter_context(tc.tile_pool(name="singles", bufs=1))
        # bufs=8 matches the 8 HWDGE sem lanes: a slot's previous writer is
        # 8 DMAs ago on the same lane, so the WAW wait is elided by FIFO
        # ordering and DMA instructions stay within their 2 sync-wait slots.
        jpool = ctx.enter_context(tc.tile_pool(name="jp", bufs=8))
        sp = ctx.enter_context(tc.tile_pool(name="sp", bufs=2))

        scur = singles.tile([CH, N], f32)
        rbuf = singles.tile([CH, N], f32)
        junk = singles.tile([CH, N], f32)
        nc.sync.dma_start(out=scur[:], in_=s0.ap())
        nc.sync.dma_start(out=rbuf[:], in_=rr.ap())

        # Absorb the load-DMA semaphores with single-output copies so the
        # fused multi-operand DVE ops below never need >1 sync-wait slot.
        warm = singles.tile([CH, 8], f32)
        nc.vector.tensor_copy(out=warm[:, 0:4], in_=scur[:, 0 : N : N // 4])
        nc.vector.tensor_copy(out=warm[:, 4:8], in_=rbuf[:, 0 : N : N // 4])

        for j in range(N):
            jt = jpool.tile([CH, N], f32, tag="jt")
            nc.sync.dma_start(out=jt[:], in_=jm.ap()[j])

            # Absorb the (possibly multi-queue) DMA semaphores with a tiny
            # single-output copy: the S2S2D2_STT struct below has only one
            # sync-wait slot, and same-engine ordering then needs no sems.
            sink = sp.tile([CH, 4], f32, tag="sink")
            nc.vector.tensor_copy(out=sink[:], in_=jt[:, 0 : N : N // 4])

            fld = sp.tile([CH, 1], f32, tag="fld")
            # junk = jt * scur ; fld = sum(junk) over free dim
            nc.vector.scalar_tensor_tensor(
                out=junk[:],
                in0=jt[:],
                scalar=1.0,
                in1=scur[:],
                op0=op.mult,
                op1=op.mult,
                accum_out=fld[:],
            )
            phi = sp.tile([CH, 1], f32, tag="phi")
            nc.vector.tensor_scalar(
                out=phi[:],
                in0=fld[:],
                scalar1=scur[:, j : j + 1],
                scalar2=rbuf[:, j : j + 1],
                op0=op.mult,
                op1=op.subtract,
            )
            sb = sp.tile([CH, 1], f32, tag="sb")
            nc.vector.scalar_tensor_tensor(
                out=sb[:],
                in0=phi[:],
                scalar=0.0,
                in1=scur[:, j : j + 1],
                op0=op.is_lt,
                op1=op.mult,
            )
            # s_j = s_j - 2*sb  (in-place elementwise)
            nc.vector.scalar_tensor_tensor(
                out=scur[:, j : j + 1],
                in0=sb[:],
                scalar=-2.0,
                in1=scur[:, j : j + 1],
                op0=op.mult,
                op1=op.add,
            )

        nc.sync.dma_start(out=so.ap(), in_=scur[:])

    nc.compile()
    return nc


def _get_nc():
    if "nc" not in _cache:
        _cache["nc"] = _build()
    return _cache["nc"]


def _run(s, h, J_sym, u, trace=False):
    from concourse.bass_utils import run_bass_kernel_spmd

    s = np.asarray(s, dtype=np.float32).reshape(R * S, N)
    h = np.asarray(h, dtype=np.float32).reshape(R * S, N)
    J = np.asarray(J_sym, dtype=np.float32).reshape(R * S, N, N)
    u = np.asarray(u, dtype=np.float32).reshape(R * S, N)

    r_eff = (-np.log(u)) - s * h  # threshold with h folded in

    in_maps = []
    for c in range(NCORES):
        lo, hi = c * CH, (c + 1) * CH
        Jc = J[lo:hi]  # [CH, N, N]
        jm = np.ascontiguousarray(Jc.transpose(1, 0, 2))  # [j, c, k]
        in_maps.append(
            {
                "jm": jm,
                "s0": np.ascontiguousarray(s[lo:hi]),
                "rr": np.ascontiguousarray(r_eff[lo:hi]),
            }
        )

    nc = _get_nc()
    res = run_bass_kernel_spmd(
        nc, in_maps, core_ids=list(range(NCORES)), trace=trace
    )
    out = np.concatenate([res.results[c]["so"] for c in range(NCORES)], axis=0)
    return out.reshape(R, S, N).astype(np.float32), res.exec_time_ns


def kernel(s, h, J_sym, u):
    out, _ = _run(s, h, J_sym, u, trace=False)
    return out


def kernel_timed(s, h, J_sym, u):
    return _run(s, h, J_sym, u, trace=True)



# revision 5
# speedup vs baseline: 1.0098x; 1.0098x over previous
"""Trainium2 Bass kernel v2 for nn_IsingModel: blocked margin-vector Gibbs sweep.

Algorithm (validated in golden.blocked_sweep_v3, 0 mismatches vs reference):
  - margin Z[u] = s_u*(h_u + field_u) - r_u for pending nodes, maintained
    incrementally; flag f_j = (Z2_j + Kd_j*f_{j-1} < 0) computed as a
    2-node tensor_tensor_scan: f = (-Kd*f_prev) is_gt Z.
  - per flag: Z row update over in-block suffix u>=t+2 (u=t+1 via scan Kd).
  - per block of B=40 nodes: packed field ops (4 nodes per [128,360] op,
    chains replicated at partition offsets 0/32/64/96) compute the next
    block's base margins from s_rep lagged one block; the lag is repaired
    by Jx accumulation ops (flags 0..31) and explicit cross rows (flags
    32..39) at block tail.

Engines: vector = chain (scan + 2 row updates) + Jx + cross rows;
         gpsimd = field ops + Z-init plumbing + spin commit;
         sync/scalar(Act) = DMA issue.
"""

import sys

if "/opt/trn_rl_repo" not in sys.path:
    sys.path.insert(0, "/opt/trn_rl_repo")

from contextlib import ExitStack

import numpy as np

R, S, N = 10, 20, 360
NCORES = 8
CH = (R * S) // NCORES  # 25 chains per core
B = 40
NB = N // B  # 9
G = 4
M = B // G  # 10 field ops per block
W = 8  # cross-window flags
BXW = B - W  # 32 flags covered by Jx ops

_cache = {}


def _build():
    import concourse.bass as bass
    import concourse.tile as tile
    from concourse import bacc, mybir

    f32 = mybir.dt.float32
    op = mybir.AluOpType

    nc = bacc.Bacc("TRN2", target_bir_lowering=False, debug=False)

    # DRAM inputs (host-prepped layouts)
    jtd = nc.dram_tensor("jtd", [NB, G, CH, M * N], f32, kind="ExternalInput")
    jxqd = nc.dram_tensor("jxqd", [NB, G, CH, M * BXW], f32, kind="ExternalInput")
    m2d = nc.dram_tensor("m2d", [NB, CH, B * B + W * B], f32, kind="ExternalInput")
    nkd_d = nc.dram_tensor("nkd", [CH, N], f32, kind="ExternalInput")
    sm2_d = nc.dram_tensor("sm2", [CH, N], f32, kind="ExternalInput")
    s0t_d = nc.dram_tensor("s0t", [CH, N], f32, kind="ExternalInput")
    srep_d = nc.dram_tensor("srep0", [128, N], f32, kind="ExternalInput")
    rr0_d = nc.dram_tensor("rr0", [128, NB * M], f32, kind="ExternalInput")
    spk_d = nc.dram_tensor("spk", [128, NB * M], f32, kind="ExternalInput")
    z0_d = nc.dram_tensor("z0", [CH, B], f32, kind="ExternalInput")
    so = nc.dram_tensor("so", [CH, N], f32, kind="ExternalOutput")

    with tile.TileContext(nc) as tc, ExitStack() as ctx:
        sg = ctx.enter_context(tc.tile_pool(name="sg", bufs=1))

        # static tiles
        nkd = sg.tile([CH, N], f32)
        sm2 = sg.tile([CH, N], f32)
        s0t = sg.tile([CH, N], f32)
        srep = sg.tile([128, N], f32)
        rr0 = sg.tile([128, NB * M], f32)
        spk = sg.tile([128, NB * M], f32)
        scur = sg.tile([CH, N], f32)

        # rings
        NJ = 3
        megaJ = [sg.tile([128, M * N], f32, name=f"megaJ{k}") for k in range(NJ)]
        megaX = [sg.tile([128, M * BXW], f32, name=f"megaX{k}") for k in range(NJ)]
        mega2 = [sg.tile([CH, B * B + W * B], f32, name=f"mega2{k}") for k in range(NJ)]
        Zt = [sg.tile([CH, B], f32, name=f"Z{k}") for k in range(2)]
        Dt = [sg.tile([128, B + 1], f32, name=f"D{k}") for k in range(2)]
        Gpk = [sg.tile([128, M], f32, name=f"Gpk{k}") for k in range(2)]
        u1t = [sg.tile([128, M], f32, name=f"u1{k}") for k in range(2)]
        t1t = [sg.tile([128, M], f32, name=f"t1{k}") for k in range(2)]
        Jxa = sg.tile([128, M], f32)
        Zpk = sg.tile([128, M], f32)
        junkf = sg.tile([128, N], f32)
        junkx = sg.tile([128, BXW], f32)
        y1 = sg.tile([CH, B], f32)
        sinkJ = [sg.tile([128, 1], f32, name=f"sinkJ{k}") for k in range(2)]
        sinkX = [sg.tile([128, 1], f32, name=f"sinkX{k}") for k in range(2)]

        # ---- prologue ----
        # memset pad rows of packed rings (pads stay 0 forever)
        for k in range(NJ):
            nc.vector.memset(megaJ[k][:], 0.0)
            nc.vector.memset(megaX[k][:], 0.0)
        nc.vector.memset(Dt[0][:], 0.0)
        nc.vector.memset(Dt[1][:], 0.0)

        nc.sync.dma_start(out=nkd[:], in_=nkd_d.ap())
        nc.sync.dma_start(out=sm2[:], in_=sm2_d.ap())
        nc.sync.dma_start(out=s0t[:], in_=s0t_d.ap())
        nc.sync.dma_start(out=srep[:], in_=srep_d.ap())
        nc.scalar.dma_start(out=rr0[:], in_=rr0_d.ap())
        nc.scalar.dma_start(out=spk[:], in_=spk_d.ap())
        nc.scalar.dma_start(out=Zt[0][:], in_=z0_d.ap())
        # block 0 patch rows + (unused) cross rows
        nc.sync.dma_start(out=mega2[0][:], in_=m2d.ap()[0])
        # field tiles for target block 1 (used during block 0)
        for g in range(G):
            eng = nc.sync if g % 2 == 0 else nc.scalar
            eng.dma_start(out=megaJ[0][32 * g : 32 * g + CH, :], in_=jtd.ap()[1, g])

        for b in range(NB):
            Z = Zt[b % 2]
            Zn = Zt[(b + 1) % 2]
            D = Dt[b % 2]
            Dn = Dt[(b + 1) % 2]
            jb = b * B
            mJ = megaJ[b % NJ]
            mJn = megaJ[(b + 1) % NJ]
            mX = megaX[(b + 1) % NJ]  # Jx tiles for target b+1, used at tail of b
            m2 = mega2[b % NJ]
            m2n = mega2[(b + 1) % NJ]

            # ---- DMA prefetch (issued early; engines alternate) ----
            if b + 2 < NB:
                for g in range(G):
                    eng = nc.sync if g % 2 == 0 else nc.scalar
                    eng.dma_start(
                        out=mJn[32 * g : 32 * g + CH, :], in_=jtd.ap()[b + 2, g]
                    )
            if b + 1 < NB:
                for g in range(G):
                    eng = nc.scalar if g % 2 == 0 else nc.sync
                    eng.dma_start(
                        out=mX[32 * g : 32 * g + CH, :], in_=jxqd.ap()[b + 1, g]
                    )
                nc.sync.dma_start(out=m2n[:], in_=m2d.ap()[b + 1])

            # ---- gpsimd: field ops for target block b+1 ----
            if b + 1 < NB:
                snk = sinkJ[b % 2]
                nc.vector.tensor_copy(out=snk[:], in_=mJ[:, 0:1])
                gp = Gpk[b % 2]
                for i in range(M):
                    nc.vector.scalar_tensor_tensor(
                        out=junkf[:],
                        in0=mJ[:, i * N : (i + 1) * N],
                        scalar=1.0,
                        in1=srep[:],
                        op0=op.mult,
                        op1=op.mult,
                        accum_out=gp[:, i : i + 1],
                    )
                t1 = t1t[b % 2]
                u1 = u1t[b % 2]
                lo, hi = (b + 1) * M, (b + 2) * M
                nc.vector.tensor_tensor(
                    out=t1[:], in0=gp[:], in1=spk[:, lo:hi], op=op.mult
                )
                nc.vector.tensor_tensor(
                    out=u1[:], in0=t1[:], in1=rr0[:, lo:hi], op=op.subtract
                )

            # ---- vector: chain (scan + row updates), tail work interleaved ----
            for p in range(B // 2):
                t = 2 * p
                nc.vector.tensor_tensor_scan(
                    out=D[0:CH, 1 + t : 3 + t],
                    data0=nkd[:, jb + t : jb + t + 2],
                    data1=Z[:, t : t + 2],
                    initial=D[0:CH, t : t + 1],
                    op0=op.mult,
                    op1=op.is_gt,
                )
                for tt in (t, t + 1):
                    if tt + 2 < B:
                        nc.vector.scalar_tensor_tensor(
                            out=Z[:, tt + 2 : B],
                            in0=m2[:, tt * B + tt + 2 : tt * B + B],
                            scalar=D[0:CH, 1 + tt : 2 + tt],
                            in1=Z[:, tt + 2 : B],
                            op0=op.mult,
                            op1=op.add,
                        )
                if b + 1 < NB:
                    if p == BXW // 2 - 1:
                        # flags 0..BXW-1 committed: replicate and run Jx
                        for g in range(1, G):
                            nc.vector.tensor_copy(
                                out=D[32 * g : 32 * g + CH, 1 : 1 + BXW],
                                in_=D[0:CH, 1 : 1 + BXW],
                            )
                        snx = sinkX[b % 2]
                        nc.vector.tensor_copy(out=snx[:], in_=mX[:, 0:1])
                        for i in range(M):
                            nc.vector.scalar_tensor_tensor(
                                out=junkx[:],
                                in0=mX[:, i * BXW : (i + 1) * BXW],
                                scalar=1.0,
                                in1=D[0:128, 1 : 1 + BXW],
                                op0=op.mult,
                                op1=op.mult,
                                accum_out=Jxa[:, i : i + 1],
                            )
                        nc.vector.tensor_tensor(
                            out=Zpk[:], in0=u1t[b % 2][:], in1=Jxa[:], op=op.add
                        )
                        for g in range(G):
                            nc.vector.tensor_copy(
                                out=Zn[:, g : B : G],
                                in_=Zpk[32 * g : 32 * g + CH, 0:M],
                            )
                    elif p >= BXW // 2:
                        # cross rows for the two flags of this pair
                        for tt in (t, t + 1):
                            widx = tt - BXW
                            nc.vector.scalar_tensor_tensor(
                                out=Zn[:, 0:B],
                                in0=m2n[:, B * B + widx * B : B * B + (widx + 1) * B],
                                scalar=D[0:CH, 1 + tt : 2 + tt],
                                in1=Zn[:, 0:B],
                                op0=op.mult,
                                op1=op.add,
                            )

            # D transition for next block
            if b + 1 < NB:
                nc.vector.tensor_copy(out=Dn[0:CH, 0:1], in_=D[0:CH, B : B + 1])

            # ---- gpsimd: commit spins ----
            nc.vector.scalar_tensor_tensor(
                out=y1[:],
                in0=D[0:CH, 1 : B + 1],
                scalar=1.0,
                in1=sm2[:, jb : jb + B],
                op0=op.mult,
                op1=op.mult,
            )
            nc.vector.tensor_tensor(
                out=scur[:, jb : jb + B],
                in0=y1[:],
                in1=s0t[:, jb : jb + B],
                op=op.add,
            )
            if b + 2 < NB:
                # srep update feeds field ops of target b+2 (run during b+1)
                for g in range(G):
                    nc.vector.tensor_copy(
                        out=srep[32 * g : 32 * g + CH, jb : jb + B],
                        in_=scur[:, jb : jb + B],
                    )

        nc.sync.dma_start(out=so.ap(), in_=scur[:])

    nc.compile()
    return nc


def _prep_core(s, h, J, r_eff):
    """Host prep for one core. s,h: [CH,N]; J: [CH,N,N]; r_eff: [CH,N]."""
    f32 = np.float32
    s0 = s.astype(f32)
    idx = np.arange(N)

    # Jss[c, j1, j2] = -2*s0[j1]*s0[j2]*J[j1,j2]
    # built lazily per slice to save memory
    def jss(c, j1s, j2s):
        return (
            -2.0 * s0[c, j1s][:, None] * s0[c, j2s][None, :] * J[c][np.ix_(j1s, j2s)]
        ).astype(f32)

    jtd = np.zeros((NB, G, CH, M * N), dtype=f32)
    jxqd = np.zeros((NB, G, CH, M * BXW), dtype=f32)
    m2d = np.zeros((NB, CH, B * B + W * B), dtype=f32)

    for bb in range(NB):
        jbb = bb * B
        nodes = jbb + 4 * np.arange(M)[:, None] + np.arange(G)[None, :]  # [M,G]
        if bb >= 1:
            for g in range(G):
                cols = nodes[:, g]  # [M]
                # jt: J[c, k, col] -> [CH, M, N]
                block = J[:, :, cols].transpose(0, 2, 1)  # [CH, M, N]
                jtd[bb, g] = block.reshape(CH, M * N)
                # jxq: Jss_cross(flag t' of bb-1, col) [CH, M, BXW]
                j1s = (bb - 1) * B + np.arange(BXW)
                for c in range(CH):
                    xb = jss(c, j1s, cols)  # [BXW, M]
                    jxqd[bb, g, c] = xb.T.reshape(M * BXW)
        # jpatch rows
        for c in range(CH):
            patch = jss(c, jbb + np.arange(B), jbb + np.arange(B))  # [B,B]
            mask = np.zeros((B, B), dtype=f32)
            for t in range(B):
                mask[t, t + 2 :] = 1.0
            m2d[bb, c, : B * B] = (patch * mask).reshape(-1)
            if bb >= 1:
                j1s = (bb - 1) * B + BXW + np.arange(W)
                rows = jss(c, j1s, jbb + np.arange(B))  # [W, B]
                rows[W - 1, 0] = 0.0
                m2d[bb, c, B * B :] = rows.reshape(-1)

    nkd = np.zeros((CH, N), dtype=f32)
    nkd[:, 1:] = (2.0 * s0[:, :-1] * s0[:, 1:] * J[:, idx[:-1], idx[1:]]).astype(f32)
    sm2 = (-2.0 * s0).astype(f32)

    srep0 = np.zeros((128, N), dtype=f32)
    rr0 = np.zeros((128, NB * M), dtype=f32)
    spk = np.zeros((128, NB * M), dtype=f32)
    for g in range(G):
        srep0[32 * g : 32 * g + CH] = s0
        # packed per-node orders: col b*M+i -> node jb+4i+g
        cols = (
            np.arange(NB)[:, None] * B + 4 * np.arange(M)[None, :] + g
        ).reshape(-1)  # [NB*M]
        rr0[32 * g : 32 * g + CH] = r_eff[:, cols]
        spk[32 * g : 32 * g + CH] = s0[:, cols]

    # z0: margins for block 0
    G0 = np.einsum("ckt,ck->ct", J[:, :, :B].astype(f32), s0, dtype=np.float32).astype(
        f32
    )
    # match device accum order closer: simple sum (validated via golden)
    G0 = (J[:, :, :B].astype(f32) * s0[:, :, None]).sum(axis=1, dtype=f32)
    z0 = (s0[:, :B] * G0 - r_eff[:, :B]).astype(f32)

    return {
        "jtd": jtd,
        "jxqd": jxqd,
        "m2d": m2d,
        "nkd": nkd,
        "sm2": sm2,
        "s0t": s0,
        "srep0": srep0,
        "rr0": rr0,
        "spk": spk,
        "z0": z0,
    }


def _get_nc():
    if "nc" not in _cache:
        _cache["nc"] = _build()
    return _cache["nc"]


def prep_all(s, h, J_sym, u):
    s = np.asarray(s, dtype=np.float32).reshape(R * S, N)
    h = np.asarray(h, dtype=np.float32).reshape(R * S, N)
    J = np.asarray(J_sym, dtype=np.float32).reshape(R * S, N, N)
    u = np.asarray(u, dtype=np.float32)
    r_eff = (-np.log(u)).reshape(R * S, N).astype(np.float32) - s * h

    in_maps = []
    for core in range(NCORES):
        lo, hi = core * CH, (core + 1) * CH
        in_maps.append(_prep_core(s[lo:hi], h[lo:hi], J[lo:hi], r_eff[lo:hi]))
    return in_maps


def _run(s, h, J_sym, u, trace=False):
    from concourse.bass_utils import run_bass_kernel_spmd

    in_maps = prep_all(s, h, J_sym, u)
    nc = _get_nc()
    res = run_bass_kernel_spmd(nc, in_maps, core_ids=list(range(NCORES)), trace=trace)
    out = np.concatenate([res.results[c]["so"] for c in range(NCORES)], axis=0)
    return out.reshape(R, S, N).astype(np.float32), res.exec_time_ns


def kernel(s, h, J_sym, u):
    out, _ = _run(s, h, J_sym, u, trace=False)
    return out


def kernel_timed(s, h, J_sym, u):
    return _run(s, h, J_sym, u, trace=True)


# revision 6
# speedup vs baseline: 1.0225x; 1.0126x over previous
"""Trainium2 Bass kernel v8: blocked margin-vector Gibbs sweep, zero-lag fields.

vs v2: the field ops for block b+1 run right after block b's spin commit, so
they see the fully updated state - the whole lag-repair machinery (Jx ops,
flag replication, cross rows, Zpk merge, D transition) disappears. The only
correction: nkd (scan adjacent coupling) is zeroed at block boundaries on the
host. Everything stays on the vector engine in one in-order stream.
"""

import sys

if "/opt/trn_rl_repo" not in sys.path:
    sys.path.insert(0, "/opt/trn_rl_repo")

from contextlib import ExitStack

import numpy as np

R, S, N = 10, 20, 360
NCORES = 8
CH = (R * S) // NCORES  # 25
B = 40
NB = N // B  # 9
G = 4
M = B // G  # 10

_cache = {}


def _build():
    import concourse.bass as bass
    import concourse.tile as tile
    from concourse import bacc, mybir

    f32 = mybir.dt.float32
    op = mybir.AluOpType

    nc = bacc.Bacc("TRN2", target_bir_lowering=False, debug=False)

    jtd = nc.dram_tensor("jtd", [NB, G, CH, M * N], f32, kind="ExternalInput")
    m2d = nc.dram_tensor("m2d", [NB, CH, B * B], f32, kind="ExternalInput")
    nkd_d = nc.dram_tensor("nkd", [CH, N], f32, kind="ExternalInput")
    sm2_d = nc.dram_tensor("sm2", [CH, N], f32, kind="ExternalInput")
    s0t_d = nc.dram_tensor("s0t", [CH, N], f32, kind="ExternalInput")
    srep_d = nc.dram_tensor("srep0", [128, N], f32, kind="ExternalInput")
    rr0_d = nc.dram_tensor("rr0", [128, NB * M], f32, kind="ExternalInput")
    spk_d = nc.dram_tensor("spk", [128, NB * M], f32, kind="ExternalInput")
    z0_d = nc.dram_tensor("z0", [CH, B], f32, kind="ExternalInput")
    so = nc.dram_tensor("so", [CH, N], f32, kind="ExternalOutput")

    with tile.TileContext(nc) as tc, ExitStack() as ctx:
        sg = ctx.enter_context(tc.tile_pool(name="sg", bufs=1))

        nkd = sg.tile([CH, N], f32)
        sm2 = sg.tile([CH, N], f32)
        s0t = sg.tile([CH, N], f32)
        srep = sg.tile([128, N], f32)
        rr0 = sg.tile([128, NB * M], f32)
        spk = sg.tile([128, NB * M], f32)
        scur = sg.tile([CH, N], f32)

        NJ = 3
        megaJ = [sg.tile([128, M * N], f32, name=f"megaJ{k}") for k in range(NJ)]
        mega2 = [sg.tile([CH, B * B], f32, name=f"mega2{k}") for k in range(NJ)]
        Zt = [sg.tile([CH, B], f32, name=f"Z{k}") for k in range(2)]
        Dt = [sg.tile([CH, B + 1], f32, name=f"D{k}") for k in range(2)]
        Gpk = sg.tile([128, M], f32)
        t1 = sg.tile([128, M], f32)
        u1 = sg.tile([128, M], f32)
        junkf = sg.tile([128, N], f32)
        y1 = sg.tile([CH, B], f32)
        sinkJ = [sg.tile([128, 1], f32, name=f"sinkJ{k}") for k in range(2)]

        # ---- prologue ----
        for k in range(NJ):
            nc.vector.memset(megaJ[k][:], 0.0)
        nc.vector.memset(Dt[0][:, 0:1], 0.0)
        nc.vector.memset(Dt[1][:, 0:1], 0.0)

        nc.sync.dma_start(out=nkd[:], in_=nkd_d.ap())
        nc.sync.dma_start(out=sm2[:], in_=sm2_d.ap())
        nc.sync.dma_start(out=s0t[:], in_=s0t_d.ap())
        nc.sync.dma_start(out=srep[:], in_=srep_d.ap())
        nc.scalar.dma_start(out=rr0[:], in_=rr0_d.ap())
        nc.scalar.dma_start(out=spk[:], in_=spk_d.ap())
        nc.scalar.dma_start(out=Zt[0][:], in_=z0_d.ap())
        nc.sync.dma_start(out=mega2[0][:], in_=m2d.ap()[0])
        for g in range(G):
            eng = nc.sync if g % 2 == 0 else nc.scalar
            eng.dma_start(out=megaJ[0][32 * g : 32 * g + CH, :], in_=jtd.ap()[1, g])

        for b in range(NB):
            Z = Zt[b % 2]
            Zn = Zt[(b + 1) % 2]
            D = Dt[b % 2]
            jb = b * B
            mJ = megaJ[b % NJ]
            mJn = megaJ[(b + 1) % NJ]
            m2 = mega2[b % NJ]
            m2n = mega2[(b + 1) % NJ]

            # ---- DMA prefetch ----
            if b + 2 < NB:
                for g in range(G):
                    eng = nc.sync if g % 2 == 0 else nc.scalar
                    eng.dma_start(
                        out=mJn[32 * g : 32 * g + CH, :], in_=jtd.ap()[b + 2, g]
                    )
            if b + 1 < NB:
                nc.sync.dma_start(out=m2n[:], in_=m2d.ap()[b + 1])

            # ---- chain: scan pairs + in-block row updates ----
            for p in range(B // 2):
                t = 2 * p
                nc.vector.tensor_tensor_scan(
                    out=D[:, 1 + t : 3 + t],
                    data0=nkd[:, jb + t : jb + t + 2],
                    data1=Z[:, t : t + 2],
                    initial=D[:, t : t + 1],
                    op0=op.mult,
                    op1=op.is_gt,
                )
                for tt in (t, t + 1):
                    if tt + 2 < B:
                        nc.vector.scalar_tensor_tensor(
                            out=Z[:, tt + 2 : B],
                            in0=m2[:, tt * B + tt + 2 : tt * B + B],
                            scalar=D[:, 1 + tt : 2 + tt],
                            in1=Z[:, tt + 2 : B],
                            op0=op.mult,
                            op1=op.add,
                        )

            # ---- commit spins, update srep, zero-lag fields for b+1 ----
            nc.vector.scalar_tensor_tensor(
                out=y1[:],
                in0=D[:, 1 : B + 1],
                scalar=1.0,
                in1=sm2[:, jb : jb + B],
                op0=op.mult,
                op1=op.mult,
            )
            nc.vector.tensor_tensor(
                out=scur[:, jb : jb + B],
                in0=y1[:],
                in1=s0t[:, jb : jb + B],
                op=op.add,
            )
            if b + 1 < NB:
                for g in range(G):
                    nc.vector.tensor_copy(
                        out=srep[32 * g : 32 * g + CH, jb : jb + B],
                        in_=scur[:, jb : jb + B],
                    )
                snk = sinkJ[b % 2]
                nc.vector.tensor_copy(out=snk[:], in_=mJ[:, 0:1])
                for i in range(M):
                    nc.vector.scalar_tensor_tensor(
                        out=junkf[:],
                        in0=mJ[:, i * N : (i + 1) * N],
                        scalar=1.0,
                        in1=srep[:],
                        op0=op.mult,
                        op1=op.mult,
                        accum_out=Gpk[:, i : i + 1],
                    )
                lo, hi = (b + 1) * M, (b + 2) * M
                nc.vector.tensor_tensor(
                    out=t1[:], in0=Gpk[:], in1=spk[:, lo:hi], op=op.mult
                )
                nc.vector.tensor_tensor(
                    out=u1[:], in0=t1[:], in1=rr0[:, lo:hi], op=op.subtract
                )
                for g in range(G):
                    nc.vector.tensor_copy(
                        out=Zn[:, g : B : G], in_=u1[32 * g : 32 * g + CH, 0:M]
                    )

        nc.sync.dma_start(out=so.ap(), in_=scur[:])

    nc.compile()
    return nc


def _prep_core(s, h, J, r_eff):
    f32 = np.float32
    s0 = s.astype(f32)
    idx = np.arange(N)

    def jss(c, j1s, j2s):
        return (
            -2.0 * s0[c, j1s][:, None] * s0[c, j2s][None, :] * J[c][np.ix_(j1s, j2s)]
        ).astype(f32)

    jtd = np.zeros((NB, G, CH, M * N), dtype=f32)
    m2d = np.zeros((NB, CH, B * B), dtype=f32)

    for bb in range(NB):
        jbb = bb * B
        nodes = jbb + 4 * np.arange(M)[:, None] + np.arange(G)[None, :]  # [M,G]
        if bb >= 1:
            for g in range(G):
                cols = nodes[:, g]
                block = J[:, :, cols].transpose(0, 2, 1)  # [CH, M, N]
                jtd[bb, g] = block.reshape(CH, M * N)
        for c in range(CH):
            patch = jss(c, jbb + np.arange(B), jbb + np.arange(B))
            mask = np.zeros((B, B), dtype=f32)
            for t in range(B):
                mask[t, t + 2 :] = 1.0
            m2d[bb, c] = (patch * mask).reshape(-1)

    nkd = np.zeros((CH, N), dtype=f32)
    nkd[:, 1:] = (2.0 * s0[:, :-1] * s0[:, 1:] * J[:, idx[:-1], idx[1:]]).astype(f32)
    nkd[:, ::B] = 0.0  # zero-lag: no cross-block adjacent coupling in the scan
    sm2 = (-2.0 * s0).astype(f32)

    srep0 = np.zeros((128, N), dtype=f32)
    rr0 = np.zeros((128, NB * M), dtype=f32)
    spk = np.zeros((128, NB * M), dtype=f32)
    for g in range(G):
        srep0[32 * g : 32 * g + CH] = s0
        cols = (
            np.arange(NB)[:, None] * B + 4 * np.arange(M)[None, :] + g
        ).reshape(-1)
        rr0[32 * g : 32 * g + CH] = r_eff[:, cols]
        spk[32 * g : 32 * g + CH] = s0[:, cols]

    G0 = (J[:, :, :B].astype(f32) * s0[:, :, None]).sum(axis=1, dtype=f32)
    z0 = (s0[:, :B] * G0 - r_eff[:, :B]).astype(f32)

    return {
        "jtd": jtd,
        "m2d": m2d,
        "nkd": nkd,
        "sm2": sm2,
        "s0t": s0,
        "srep0": srep0,
        "rr0": rr0,
        "spk": spk,
        "z0": z0,
    }


def _get_nc():
    if "nc" not in _cache:
        _cache["nc"] = _build()
    return _cache["nc"]


def prep_all(s, h, J_sym, u):
    s = np.asarray(s, dtype=np.float32).reshape(R * S, N)
    h = np.asarray(h, dtype=np.float32).reshape(R * S, N)
    J = np.asarray(J_sym, dtype=np.float32).reshape(R * S, N, N)
    u = np.asarray(u, dtype=np.float32)
    r_eff = (-np.log(u)).reshape(R * S, N).astype(np.float32) - s * h

    in_maps = []
    for core in range(NCORES):
        lo, hi = core * CH, (core + 1) * CH
        in_maps.append(_prep_core(s[lo:hi], h[lo:hi], J[lo:hi], r_eff[lo:hi]))
    return in_maps


def _run(s, h, J_sym, u, trace=False):
    from concourse.bass_utils import run_bass_kernel_spmd

    in_maps = prep_all(s, h, J_sym, u)
    nc = _get_nc()
    res = run_bass_kernel_spmd(nc, in_maps, core_ids=list(range(NCORES)), trace=trace)
    out = np.concatenate([res.results[c]["so"] for c in range(NCORES)], axis=0)
    return out.reshape(R, S, N).astype(np.float32), res.exec_time_ns


def kernel(s, h, J_sym, u):
    out, _ = _run(s, h, J_sym, u, trace=False)
    return out


def kernel_timed(s, h, J_sym, u):
    return _run(s, h, J_sym, u, trace=True)
